# revision 4
# baseline (speedup 1.0000x reference)
"""Trainium2 Bass kernel: MHSA with multi-head relative position embedding.

Sharding: data-parallel over batch — 16 batches / 8 cores = 2 per core.

Attention: 16 units of (head, batch), software-pipelined kt-rounds:
    sc(kt) [2 mm] -> vmm(kt-1) [2 mm] -> exp(kt) [ACT, one [112,784] op]
    -> bias-mult(kt) [DVE, one [112,784] bf16 2x op]
PSUM: 'big' tag ring, 3 x [128,784] f32 slots (6 banks) serves scores,
qkv-projection and out-projection psums; 'ot' tag ring, 1 x [65,784] f32
(2 banks) holds the attn-out accumulator.  The 3-deep scores ring lets the
PE run 2-3 rounds ahead of the ACT exp stream, so neither engine ever
head-of-line blocks on the other, and qkv/proj filler tiles slot into the
same ring at unit boundaries to keep the PE dense (HAM stays at full clock).

The attn-out accumulator is drained to SBUF with one DVE copy the moment
the unit ends (freeing its only PSUM slot); the reciprocal-normalize of
unit u is emitted in the middle of unit u+1 where the DVE has slack.
"""

import numpy as np
import ml_dtypes

B, HH, WW, C = 16, 28, 28, 512
N = HH * WW            # 784 tokens
HEADS, KD = 8, 64
NCORES, BPC = 8, 2     # 8 cores, 2 batches per core
NT, TP = 7, 112        # 784 = 7 tiles of 112 (k / token tiling)
CHUNKS = [(0, 512), (512, 272)]   # matmul N-chunks (PSUM bank = 512 fp32)
CT = 4                 # contraction tiles of 128 over C=512

_CACHE = {}


def _rel_index():
    # Faithful to reference._relative_position_index: token r -> (r%28, r//28)
    t = np.arange(N)
    c0, c1 = t % HH, t // HH
    return ((c0[:, None] - c0[None, :] + HH - 1)
            + (c1[:, None] - c1[None, :] + WW - 1) * (2 * HH - 1))  # [q, k]


def build_nc():
    if 'nc' in _CACHE:
        return _CACHE['nc']
    from contextlib import ExitStack
    import concourse.bacc as bacc
    import concourse.mybir as mybir
    import concourse.tile as tile
    from concourse.alu_op_type import AluOpType

    f32 = mybir.dt.float32
    bf16 = mybir.dt.bfloat16
    EXP = mybir.ActivationFunctionType.Exp

    nc = bacc.Bacc("TRN2", debug=False, enable_asserts=False)
    xT_d = nc.dram_tensor("xT", [BPC, C, N], bf16, kind="ExternalInput").ap()
    wqkv_d = nc.dram_tensor("wqkv", [C, 3 * C], bf16, kind="ExternalInput").ap()
    wout_d = nc.dram_tensor("wout", [C, C], bf16, kind="ExternalInput").ap()
    bias_d = nc.dram_tensor("biasT", [HEADS, TP, NT, N], bf16,
                            kind="ExternalInput").ap()
    out_d = nc.dram_tensor("out", [BPC, N, C], f32, kind="ExternalOutput").ap()

    with tile.TileContext(nc) as tc, ExitStack() as ctx:
        persist = ctx.enter_context(tc.tile_pool(name="persist", bufs=1))
        xT_pool = ctx.enter_context(tc.tile_pool(name="xTp", bufs=8))
        bias_pool = ctx.enter_context(tc.tile_pool(name="biasp", bufs=6))
        eraw_pool = ctx.enter_context(tc.tile_pool(name="erp", bufs=6))
        e_pool = ctx.enter_context(tc.tile_pool(name="ep", bufs=6))
        o_pool = ctx.enter_context(tc.tile_pool(name="op", bufs=3))
        r_pool = ctx.enter_context(tc.tile_pool(name="rp", bufs=3))
        osb_pool = ctx.enter_context(tc.tile_pool(name="osbp", bufs=3))
        big_psum = ctx.enter_context(tc.tile_pool(name="bigp", bufs=3,
                                                  space="PSUM"))
        ot_psum = ctx.enter_context(tc.tile_pool(name="otp", bufs=1,
                                                 space="PSUM"))

        qkT, vsb, attnT, bias_sb = {}, {}, {}, {}
        wqkv_sb, wout_sb = [None] * CT, [None] * CT
        for b in range(BPC):
            for fi in range(CT):
                attnT[b, fi] = persist.tile(
                    [128, N], bf16, tag=f"attnT{b}_{fi}", name=f"attnT{b}_{fi}")

        xts = {b: [None] * CT for b in range(BPC)}

        # DMA order: interleave wqkv/xT per contraction slice so the first
        # qk matmul can start after ~0.8MB instead of after all inputs.
        for ci in range(CT):
            w = persist.tile([128, 3 * C], bf16, tag=f"wqkv{ci}",
                             name=f"wqkv{ci}")
            nc.sync.dma_start(w, wqkv_d[ci * 128:(ci + 1) * 128, :])
            wqkv_sb[ci] = w
            for b in range(BPC):
                xt = xT_pool.tile([128, N], bf16, tag="xT", name=f"xT{b}_{ci}")
                nc.sync.dma_start(xt, xT_d[b, ci * 128:(ci + 1) * 128, :])
                xts[b][ci] = xt

        def load_wout():
            for ci in range(CT):
                w = persist.tile([128, C], bf16, tag=f"wout{ci}",
                                 name=f"wout{ci}")
                nc.sync.dma_start(w, wout_d[ci * 128:(ci + 1) * 128, :])
                wout_sb[ci] = w

        def load_bias(h):
            bt = bias_pool.tile([TP, NT, N], bf16, tag="bias", name=f"bias{h}")
            nc.sync.dma_start(bt, bias_d[h])
            bias_sb[h] = bt

        def emit_qk_tile(b, ft, on_act):
            dst = persist.tile([128, N], bf16, tag=f"qkT{b}_{ft}",
                               name=f"qkT{b}_{ft}")
            qkT[b, ft] = dst
            ps = big_psum.tile([128, N], f32, tag="big", name=f"pj{b}_{ft}")
            for (c0w, cw) in CHUNKS:
                for ci in range(CT):
                    nc.tensor.matmul(
                        ps[:, c0w:c0w + cw],
                        wqkv_sb[ci][:, ft * 128:(ft + 1) * 128],
                        xts[b][ci][:, c0w:c0w + cw],
                        start=(ci == 0), stop=(ci == CT - 1))
            if on_act:
                nc.scalar.copy(dst, ps)     # prefix: ACT idle there
            else:
                nc.vector.tensor_copy(dst, ps)

        def emit_v_tile(b, t):
            vt = persist.tile([TP, HEADS, KD + 1], bf16, tag=f"v{b}_{t}",
                              name=f"v{b}_{t}")
            vsb[b, t] = vt
            ps = big_psum.tile([TP, C], f32, tag="big", name=f"pv{b}_{t}")
            for ci in range(CT):
                nc.tensor.matmul(
                    ps, xts[b][ci][:, t * TP:(t + 1) * TP],
                    wqkv_sb[ci][:, 2 * C:3 * C],
                    start=(ci == 0), stop=(ci == CT - 1))
            nc.vector.tensor_copy(
                vt[:, :, 0:KD], ps.rearrange("p (h d) -> p h d", h=HEADS))
            nc.vector.memset(vt[:, :, KD:KD + 1], 1.0)

        # deferred reciprocal-normalize of the previous unit (emitted where
        # the next unit's DVE stream has slack)
        pending_norm = [None]

        def flush_norm():
            if pending_norm[0] is not None:
                fn, pending_norm[0] = pending_norm[0], None
                fn()

        # ---- attention for one (head, batch) unit ----
        def attention_unit(h, b):
            hp, r0 = h // 2, (h % 2) * 64
            with nc.named_scope(f"attn{h}_{b}"):
                ot = ot_psum.tile([KD + 1, N], f32, tag="ot",
                                  name=f"ot{h}_{b}")
                esbs = {}

                def emit_vmm(kt):
                    for (c0w, cw) in CHUNKS:
                        nc.tensor.matmul(
                            ot[:, c0w:c0w + cw],
                            vsb[b, kt][:, h:h + 1, :],
                            esbs[kt][:, c0w:c0w + cw],
                            start=(kt == 0), stop=(kt == NT - 1))

                for kt in range(NT):
                    scp = big_psum.tile([TP, N], f32, tag="big",
                                        name=f"sc{h}_{b}_{kt}")
                    for (c0w, cw) in CHUNKS:
                        nc.tensor.matmul(
                            scp[:, c0w:c0w + cw],
                            qkT[b, 4 + hp][r0:r0 + 64, kt * TP:(kt + 1) * TP],
                            qkT[b, hp][r0:r0 + 64, c0w:c0w + cw],
                            start=True, stop=True)
                    # v-matmul for kt-1 AFTER scores(kt): the PE stream never
                    # parks behind ops whose ACT/DVE deps aren't done yet.
                    if kt >= 1:
                        emit_vmm(kt - 1)
                    if kt == 2:
                        flush_norm()
                    eraw = eraw_pool.tile([TP, N], bf16, tag="eraw",
                                          name=f"er{h}_{b}_{kt}")
                    nc.scalar.activation(eraw, scp, EXP)
                    esb = e_pool.tile([TP, N], bf16, tag="e",
                                      name=f"e{h}_{b}_{kt}")
                    nc.vector.tensor_tensor(
                        esb, eraw, bias_sb[h][:, kt, :], AluOpType.mult)
                    esbs[kt] = esb
                emit_vmm(NT - 1)
                # drain the accumulator to SBUF at once — frees the single
                # 'ot' PSUM slot after one DVE op instead of after the whole
                # reciprocal chain
                ofull = o_pool.tile([KD + 1, N], f32, tag="ofull",
                                    name=f"of{h}_{b}")
                nc.vector.tensor_copy(ofull, ot)

                def finish(ofull=ofull, b=b, hp=hp, r0=r0):
                    # custom-DVE recip must see base-partition 0 (its ucode
                    # does not honor a partition-offset src on HW), so hop
                    # the sum-exp row through a dedicated [1, N] tile first
                    srow = r_pool.tile([1, N], f32, tag="srow")
                    nc.vector.tensor_copy(srow, ofull[KD:KD + 1, :])
                    rrow = r_pool.tile([1, N], f32, tag="rrow")
                    nc.vector.reciprocal_approx_fast(rrow, srow)
                    rb = r_pool.tile([64, N], f32, tag="rb")
                    nc.gpsimd.partition_broadcast(rb, rrow)
                    nc.vector.tensor_tensor(
                        attnT[b, hp][r0:r0 + 64, :], ofull[0:KD, :], rb,
                        AluOpType.mult)
                pending_norm[0] = finish

        def emit_proj_tile(b, t, copy_on_act):
            ps = big_psum.tile([TP, C], f32, tag="big", name=f"pr{b}_{t}")
            for fi in range(CT):
                nc.tensor.matmul(
                    ps, attnT[b, fi][:, t * TP:(t + 1) * TP], wout_sb[fi],
                    start=(fi == 0), stop=(fi == CT - 1))
            osb = osb_pool.tile([TP, C], f32, tag="osb")
            if copy_on_act:
                nc.scalar.copy(osb, ps)
            else:
                nc.vector.tensor_copy(osb, ps)
            nc.sync.dma_start(out_d[b, t * TP:(t + 1) * TP, :], osb)

        # ---- master schedule ----
        load_bias(0)
        load_bias(1)
        with nc.named_scope("qkv_prefix"):
            emit_qk_tile(0, 0, on_act=True)
            emit_qk_tile(0, 4, on_act=True)
            for t in range(NT):
                emit_v_tile(0, t)
        load_wout()

        # units: heads 0-3 on b0, then b1; heads 4-7 on b0, then b1.
        units = ([(h, 0) for h in range(4)] + [(h, 1) for h in range(4)]
                 + [(h, 0) for h in range(4, 8)] + [(h, 1) for h in range(4, 8)])
        # PE filler + bias prefetch at each unit boundary (index = after
        # unit i+1), placed just-in-time for their consumer units.
        boundary = {
            0: [('qk', 0, 1), ('qk', 0, 5), ('bias', 2)],
            1: [('qk', 1, 0), ('v', 1, 0), ('v', 1, 1), ('bias', 3)],
            2: [('qk', 1, 4), ('v', 1, 2), ('v', 1, 3), ('bias', 4)],
            3: [('v', 1, 4), ('v', 1, 5), ('v', 1, 6), ('bias', 5)],
            4: [('qk', 1, 1), ('qk', 1, 5), ('bias', 6)],
            5: [('qk', 0, 2), ('qk', 0, 6), ('bias', 7)],
            6: [('qk', 1, 2), ('qk', 1, 6)],
            7: [('qk', 0, 3), ('qk', 0, 7)],
            8: [('qk', 1, 3), ('qk', 1, 7)],
            12: [('proj', 0, 0), ('proj', 0, 1)],
            13: [('proj', 0, 2), ('proj', 0, 3)],
            14: [('proj', 0, 4), ('proj', 0, 5), ('proj', 0, 6)],
        }
        for ui, (h, b) in enumerate(units):
            attention_unit(h, b)
            for f in boundary.get(ui, []):
                kind = f[0]
                if kind == 'qk':
                    with nc.named_scope(f"qk_fill_b{f[1]}_{f[2]}"):
                        emit_qk_tile(f[1], f[2], on_act=False)
                elif kind == 'v':
                    with nc.named_scope(f"v_fill_b{f[1]}_{f[2]}"):
                        emit_v_tile(f[1], f[2])
                elif kind == 'proj':
                    with nc.named_scope(f"proj_b{f[1]}_t{f[2]}"):
                        emit_proj_tile(f[1], f[2], copy_on_act=False)
                else:
                    load_bias(f[1])
        flush_norm()

        # ---- remaining output projection (tail) ----
        for t in range(NT):
            with nc.named_scope(f"proj_b1_t{t}"):
                emit_proj_tile(1, t, copy_on_act=(t % 2 == 0))

    nc.compile()
    _CACHE['nc'] = nc
    return nc


def host_prep(x, w_qkv, pos_table, w_out):
    x = np.asarray(x, np.float32).reshape(B, N, C)
    wq = np.array(np.asarray(w_qkv, np.float32), copy=True)
    wq[:, :C] *= np.float32(1.0 / np.sqrt(KD))
    wq_bf = wq.astype(ml_dtypes.bfloat16)
    idx = _rel_index()
    expb = np.exp(np.asarray(pos_table, np.float32)[:, idx].transpose(0, 2, 1))
    # [h, k, q] -> [h, p, kt, q] with k = kt*112 + p
    biasT = np.ascontiguousarray(
        expb.reshape(HEADS, NT, TP, N).transpose(0, 2, 1, 3)
    ).astype(ml_dtypes.bfloat16)
    wout = np.ascontiguousarray(np.asarray(w_out, np.float32)).astype(
        ml_dtypes.bfloat16)
    in_maps = []
    for c in range(NCORES):
        xT = np.ascontiguousarray(
            x[c * BPC:(c + 1) * BPC].transpose(0, 2, 1)).astype(
                ml_dtypes.bfloat16)  # [2, 512, 784]
        in_maps.append({"xT": xT, "wqkv": wq_bf, "wout": wout, "biasT": biasT})
    return in_maps


def run(in_maps, trace=False, trace_cores=None):
    import concourse.bass_utils as bass_utils
    nc = build_nc()
    return bass_utils.run_bass_kernel_spmd(
        nc, in_maps, core_ids=list(range(NCORES)),
        trace=trace, trace_cores=trace_cores)


def kernel(x, w_qkv, pos_table, w_out):
    in_maps = host_prep(x, w_qkv, pos_table, w_out)
    res = run(in_maps)
    out = np.stack([r["out"] for r in res.results])    # [8, 2, 784, 512]
    return np.ascontiguousarray(out.reshape(B, HH, WW, C)).astype(np.float32)


# revision 5
# speedup vs baseline: 1.0228x; 1.0228x over previous
"""Trainium2 Bass kernel: MHSA with multi-head relative position embedding.

Sharding: data-parallel over batch — 16 batches / 8 cores = 2 per core.

Attention: 16 units of (head, batch), software-pipelined kt-rounds:
    sc(kt) [2 mm] -> vmm(kt-1) [2 mm] -> exp(kt) [ACT, one [112,784] op]
    -> bias-mult(kt) [DVE, one [112,784] bf16 2x op]
PSUM: 'big' tag ring, 3 x [128,784] f32 slots (6 banks) serves scores,
qkv-projection and out-projection psums; 'ot' tag ring, 1 x [65,784] f32
(2 banks) holds the attn-out accumulator.  The 3-deep scores ring lets the
PE run 2-3 rounds ahead of the ACT exp stream, so neither engine ever
head-of-line blocks on the other, and qkv/proj filler tiles slot into the
same ring at unit boundaries to keep the PE dense (HAM stays at full clock).

The attn-out accumulator is drained to SBUF with one DVE copy the moment
the unit ends (freeing its only PSUM slot); the reciprocal-normalize of
unit u is emitted in the middle of unit u+1 where the DVE has slack.
"""

import numpy as np
import ml_dtypes

B, HH, WW, C = 16, 28, 28, 512
N = HH * WW            # 784 tokens
HEADS, KD = 8, 64
NCORES, BPC = 8, 2     # 8 cores, 2 batches per core
NT, TP = 7, 112        # 784 = 7 tiles of 112 (k / token tiling)
CHUNKS = [(0, 512), (512, 272)]   # matmul N-chunks (PSUM bank = 512 fp32)
CT = 4                 # contraction tiles of 128 over C=512

_CACHE = {}


def _rel_index():
    # Faithful to reference._relative_position_index: token r -> (r%28, r//28)
    t = np.arange(N)
    c0, c1 = t % HH, t // HH
    return ((c0[:, None] - c0[None, :] + HH - 1)
            + (c1[:, None] - c1[None, :] + WW - 1) * (2 * HH - 1))  # [q, k]


def build_nc():
    if 'nc' in _CACHE:
        return _CACHE['nc']
    from contextlib import ExitStack
    import concourse.bacc as bacc
    import concourse.mybir as mybir
    import concourse.tile as tile
    from concourse.alu_op_type import AluOpType

    f32 = mybir.dt.float32
    bf16 = mybir.dt.bfloat16
    EXP = mybir.ActivationFunctionType.Exp

    nc = bacc.Bacc("TRN2", debug=False, enable_asserts=False)
    xT_d = nc.dram_tensor("xT", [BPC, C, N], bf16, kind="ExternalInput").ap()
    wqkv_d = nc.dram_tensor("wqkv", [C, 3 * C], bf16, kind="ExternalInput").ap()
    wout_d = nc.dram_tensor("wout", [C, C], bf16, kind="ExternalInput").ap()
    bias_d = nc.dram_tensor("biasT", [HEADS, TP, NT, N], bf16,
                            kind="ExternalInput").ap()
    out_d = nc.dram_tensor("out", [BPC, N, C], f32, kind="ExternalOutput").ap()

    with tile.TileContext(nc) as tc, ExitStack() as ctx:
        persist = ctx.enter_context(tc.tile_pool(name="persist", bufs=1))
        xT_pool = ctx.enter_context(tc.tile_pool(name="xTp", bufs=8))
        bias_pool = ctx.enter_context(tc.tile_pool(name="biasp", bufs=6))
        eraw_pool = ctx.enter_context(tc.tile_pool(name="erp", bufs=6))
        e_pool = ctx.enter_context(tc.tile_pool(name="ep", bufs=6))
        o_pool = ctx.enter_context(tc.tile_pool(name="op", bufs=3))
        r_pool = ctx.enter_context(tc.tile_pool(name="rp", bufs=3))
        osb_pool = ctx.enter_context(tc.tile_pool(name="osbp", bufs=3))
        big_psum = ctx.enter_context(tc.tile_pool(name="bigp", bufs=3,
                                                  space="PSUM"))
        ot_psum = ctx.enter_context(tc.tile_pool(name="otp", bufs=1,
                                                 space="PSUM"))

        qkT, vsb, attnT, bias_sb = {}, {}, {}, {}
        wqkv_sb, wout_sb = [None] * CT, [None] * CT
        for b in range(BPC):
            for fi in range(CT):
                attnT[b, fi] = persist.tile(
                    [128, N], bf16, tag=f"attnT{b}_{fi}", name=f"attnT{b}_{fi}")

        xts = {b: [None] * CT for b in range(BPC)}

        # DMA order: wqkv + xT(b0) first (first qk matmul unblocks after
        # ~3MB), then bias h0/h1, then xT(b1) — the prefix is DMA-bound,
        # so only unit-1's inputs go up front.
        for ci in range(CT):
            w = persist.tile([128, 3 * C], bf16, tag=f"wqkv{ci}",
                             name=f"wqkv{ci}")
            nc.sync.dma_start(w, wqkv_d[ci * 128:(ci + 1) * 128, :])
            wqkv_sb[ci] = w
            xt = xT_pool.tile([128, N], bf16, tag="xT", name=f"xT0_{ci}")
            nc.sync.dma_start(xt, xT_d[0, ci * 128:(ci + 1) * 128, :])
            xts[0][ci] = xt

        def load_xt1():
            for ci in range(CT):
                xt = xT_pool.tile([128, N], bf16, tag="xT", name=f"xT1_{ci}")
                nc.sync.dma_start(xt, xT_d[1, ci * 128:(ci + 1) * 128, :])
                xts[1][ci] = xt

        def load_wout():
            for ci in range(CT):
                w = persist.tile([128, C], bf16, tag=f"wout{ci}",
                                 name=f"wout{ci}")
                nc.sync.dma_start(w, wout_d[ci * 128:(ci + 1) * 128, :])
                wout_sb[ci] = w

        def load_bias(h):
            bt = bias_pool.tile([TP, NT, N], bf16, tag="bias", name=f"bias{h}")
            nc.sync.dma_start(bt, bias_d[h])
            bias_sb[h] = bt

        def emit_qk_tile(b, ft, on_act):
            dst = persist.tile([128, N], bf16, tag=f"qkT{b}_{ft}",
                               name=f"qkT{b}_{ft}")
            qkT[b, ft] = dst
            ps = big_psum.tile([128, N], f32, tag="big", name=f"pj{b}_{ft}")
            for (c0w, cw) in CHUNKS:
                for ci in range(CT):
                    nc.tensor.matmul(
                        ps[:, c0w:c0w + cw],
                        wqkv_sb[ci][:, ft * 128:(ft + 1) * 128],
                        xts[b][ci][:, c0w:c0w + cw],
                        start=(ci == 0), stop=(ci == CT - 1))
            if on_act:
                nc.scalar.copy(dst, ps)     # prefix: ACT idle there
            else:
                nc.vector.tensor_copy(dst, ps)

        def emit_v_tile(b, t):
            vt = persist.tile([TP, HEADS, KD + 1], bf16, tag=f"v{b}_{t}",
                              name=f"v{b}_{t}")
            vsb[b, t] = vt
            ps = big_psum.tile([TP, C], f32, tag="big", name=f"pv{b}_{t}")
            for ci in range(CT):
                nc.tensor.matmul(
                    ps, xts[b][ci][:, t * TP:(t + 1) * TP],
                    wqkv_sb[ci][:, 2 * C:3 * C],
                    start=(ci == 0), stop=(ci == CT - 1))
            nc.vector.tensor_copy(
                vt[:, :, 0:KD], ps.rearrange("p (h d) -> p h d", h=HEADS))
            nc.vector.memset(vt[:, :, KD:KD + 1], 1.0)

        # deferred reciprocal-normalize of the previous unit (emitted where
        # the next unit's DVE stream has slack)
        pending_norm = [None]

        def flush_norm():
            if pending_norm[0] is not None:
                fn, pending_norm[0] = pending_norm[0], None
                fn()

        # ---- attention for one (head, batch) unit ----
        def attention_unit(h, b):
            hp, r0 = h // 2, (h % 2) * 64
            with nc.named_scope(f"attn{h}_{b}"):
                ot = ot_psum.tile([KD + 1, N], f32, tag="ot",
                                  name=f"ot{h}_{b}")
                esbs = {}

                def emit_vmm(kt):
                    for (c0w, cw) in CHUNKS:
                        nc.tensor.matmul(
                            ot[:, c0w:c0w + cw],
                            vsb[b, kt][:, h:h + 1, :],
                            esbs[kt][:, c0w:c0w + cw],
                            start=(kt == 0), stop=(kt == NT - 1))

                for kt in range(NT):
                    scp = big_psum.tile([TP, N], f32, tag="big",
                                        name=f"sc{h}_{b}_{kt}")
                    for (c0w, cw) in CHUNKS:
                        nc.tensor.matmul(
                            scp[:, c0w:c0w + cw],
                            qkT[b, 4 + hp][r0:r0 + 64, kt * TP:(kt + 1) * TP],
                            qkT[b, hp][r0:r0 + 64, c0w:c0w + cw],
                            start=True, stop=True)
                    # v-matmul for kt-1 AFTER scores(kt): the PE stream never
                    # parks behind ops whose ACT/DVE deps aren't done yet.
                    if kt >= 1:
                        emit_vmm(kt - 1)
                    if kt == 2:
                        flush_norm()
                    eraw = eraw_pool.tile([TP, N], bf16, tag="eraw",
                                          name=f"er{h}_{b}_{kt}")
                    nc.scalar.activation(eraw, scp, EXP)
                    esb = e_pool.tile([TP, N], bf16, tag="e",
                                      name=f"e{h}_{b}_{kt}")
                    nc.vector.tensor_tensor(
                        esb, eraw, bias_sb[h][:, kt, :], AluOpType.mult)
                    esbs[kt] = esb
                emit_vmm(NT - 1)
                # drain the accumulator to SBUF at once — frees the single
                # 'ot' PSUM slot after one DVE op instead of after the whole
                # reciprocal chain
                ofull = o_pool.tile([KD + 1, N], f32, tag="ofull",
                                    name=f"of{h}_{b}")
                nc.vector.tensor_copy(ofull, ot)

                def finish(ofull=ofull, b=b, hp=hp, r0=r0):
                    # custom-DVE recip must see base-partition 0 (its ucode
                    # does not honor a partition-offset src on HW), so hop
                    # the sum-exp row through a dedicated [1, N] tile first
                    srow = r_pool.tile([1, N], f32, tag="srow")
                    nc.vector.tensor_copy(srow, ofull[KD:KD + 1, :])
                    rrow = r_pool.tile([1, N], f32, tag="rrow")
                    nc.vector.reciprocal_approx_fast(rrow, srow)
                    rb = r_pool.tile([64, N], f32, tag="rb")
                    nc.gpsimd.partition_broadcast(rb, rrow)
                    nc.vector.tensor_tensor(
                        attnT[b, hp][r0:r0 + 64, :], ofull[0:KD, :], rb,
                        AluOpType.mult)
                pending_norm[0] = finish

        def emit_proj_tile(b, t, copy_on_act):
            ps = big_psum.tile([TP, C], f32, tag="big", name=f"pr{b}_{t}")
            for fi in range(CT):
                nc.tensor.matmul(
                    ps, attnT[b, fi][:, t * TP:(t + 1) * TP], wout_sb[fi],
                    start=(fi == 0), stop=(fi == CT - 1))
            osb = osb_pool.tile([TP, C], f32, tag="osb")
            if copy_on_act:
                nc.scalar.copy(osb, ps)
            else:
                nc.vector.tensor_copy(osb, ps)
            nc.sync.dma_start(out_d[b, t * TP:(t + 1) * TP, :], osb)

        # ---- master schedule ----
        load_bias(0)
        load_bias(1)
        load_xt1()
        with nc.named_scope("qkv_prefix"):
            emit_qk_tile(0, 0, on_act=True)
            emit_qk_tile(0, 4, on_act=True)
            for t in range(NT):
                emit_v_tile(0, t)
        load_wout()

        # units: heads 0-3 on b0, then b1; heads 4-7 on b0, then b1.
        units = ([(h, 0) for h in range(4)] + [(h, 1) for h in range(4)]
                 + [(h, 0) for h in range(4, 8)] + [(h, 1) for h in range(4, 8)])
        # PE filler + bias prefetch at each unit boundary (index = after
        # unit i+1), placed just-in-time for their consumer units.
        boundary = {
            0: [('qk', 0, 1), ('qk', 0, 5), ('bias', 2)],
            1: [('qk', 1, 0), ('v', 1, 0), ('v', 1, 1), ('bias', 3)],
            2: [('qk', 1, 4), ('v', 1, 2), ('v', 1, 3), ('bias', 4)],
            3: [('v', 1, 4), ('v', 1, 5), ('v', 1, 6), ('bias', 5)],
            4: [('qk', 1, 1), ('qk', 1, 5), ('bias', 6)],
            5: [('qk', 0, 2), ('qk', 0, 6), ('bias', 7)],
            6: [('qk', 1, 2), ('qk', 1, 6)],
            7: [('qk', 0, 3), ('qk', 0, 7)],
            8: [('qk', 1, 3), ('qk', 1, 7)],
            12: [('proj', 0, 0), ('proj', 0, 1)],
            13: [('proj', 0, 2), ('proj', 0, 3)],
            14: [('proj', 0, 4), ('proj', 0, 5), ('proj', 0, 6)],
        }
        for ui, (h, b) in enumerate(units):
            attention_unit(h, b)
            for f in boundary.get(ui, []):
                kind = f[0]
                if kind == 'qk':
                    with nc.named_scope(f"qk_fill_b{f[1]}_{f[2]}"):
                        emit_qk_tile(f[1], f[2], on_act=False)
                elif kind == 'v':
                    with nc.named_scope(f"v_fill_b{f[1]}_{f[2]}"):
                        emit_v_tile(f[1], f[2])
                elif kind == 'proj':
                    with nc.named_scope(f"proj_b{f[1]}_t{f[2]}"):
                        emit_proj_tile(f[1], f[2], copy_on_act=False)
                else:
                    load_bias(f[1])
        flush_norm()

        # ---- remaining output projection (tail) ----
        for t in range(NT):
            with nc.named_scope(f"proj_b1_t{t}"):
                emit_proj_tile(1, t, copy_on_act=(t % 2 == 0))

    nc.compile()
    _CACHE['nc'] = nc
    return nc


def host_prep(x, w_qkv, pos_table, w_out):
    x = np.asarray(x, np.float32).reshape(B, N, C)
    wq = np.array(np.asarray(w_qkv, np.float32), copy=True)
    wq[:, :C] *= np.float32(1.0 / np.sqrt(KD))
    wq_bf = wq.astype(ml_dtypes.bfloat16)
    idx = _rel_index()
    expb = np.exp(np.asarray(pos_table, np.float32)[:, idx].transpose(0, 2, 1))
    # [h, k, q] -> [h, p, kt, q] with k = kt*112 + p
    biasT = np.ascontiguousarray(
        expb.reshape(HEADS, NT, TP, N).transpose(0, 2, 1, 3)
    ).astype(ml_dtypes.bfloat16)
    wout = np.ascontiguousarray(np.asarray(w_out, np.float32)).astype(
        ml_dtypes.bfloat16)
    in_maps = []
    for c in range(NCORES):
        xT = np.ascontiguousarray(
            x[c * BPC:(c + 1) * BPC].transpose(0, 2, 1)).astype(
                ml_dtypes.bfloat16)  # [2, 512, 784]
        in_maps.append({"xT": xT, "wqkv": wq_bf, "wout": wout, "biasT": biasT})
    return in_maps


def run(in_maps, trace=False, trace_cores=None):
    import concourse.bass_utils as bass_utils
    nc = build_nc()
    return bass_utils.run_bass_kernel_spmd(
        nc, in_maps, core_ids=list(range(NCORES)),
        trace=trace, trace_cores=trace_cores)


def kernel(x, w_qkv, pos_table, w_out):
    in_maps = host_prep(x, w_qkv, pos_table, w_out)
    res = run(in_maps)
    out = np.stack([r["out"] for r in res.results])    # [8, 2, 784, 512]
    return np.ascontiguousarray(out.reshape(B, HH, WW, C)).astype(np.float32)


# revision 6
# speedup vs baseline: 1.0245x; 1.0016x over previous
"""Trainium2 Bass kernel: MHSA with multi-head relative position embedding.

Sharding: data-parallel over batch — 16 batches / 8 cores = 2 per core.

Attention: 16 units of (head, batch), software-pipelined kt-rounds:
    sc(kt) [2 mm] -> vmm(kt-1) [2 mm] -> exp(kt) [ACT, one [112,784] op]
    -> bias-mult(kt) [DVE, one [112,784] bf16 2x op]
PSUM: 'big' tag ring, 3 x [128,784] f32 slots (6 banks) serves scores,
qkv-projection and out-projection psums; 'ot' tag ring, 1 x [65,784] f32
(2 banks) holds the attn-out accumulator.  The 3-deep scores ring lets the
PE run 2-3 rounds ahead of the ACT exp stream, so neither engine ever
head-of-line blocks on the other, and qkv/proj filler tiles slot into the
same ring at unit boundaries to keep the PE dense (HAM stays at full clock).

The attn-out accumulator is drained to SBUF with one DVE copy the moment
the unit ends (freeing its only PSUM slot); the reciprocal-normalize of
unit u is emitted in the middle of unit u+1 where the DVE has slack.
"""

import numpy as np
import ml_dtypes

B, HH, WW, C = 16, 28, 28, 512
N = HH * WW            # 784 tokens
HEADS, KD = 8, 64
NCORES, BPC = 8, 2     # 8 cores, 2 batches per core
NT, TP = 7, 112        # 784 = 7 tiles of 112 (k / token tiling)
CHUNKS = [(0, 512), (512, 272)]   # matmul N-chunks (PSUM bank = 512 fp32)
CT = 4                 # contraction tiles of 128 over C=512

_CACHE = {}


def _rel_index():
    # Faithful to reference._relative_position_index: token r -> (r%28, r//28)
    t = np.arange(N)
    c0, c1 = t % HH, t // HH
    return ((c0[:, None] - c0[None, :] + HH - 1)
            + (c1[:, None] - c1[None, :] + WW - 1) * (2 * HH - 1))  # [q, k]


def build_nc():
    if 'nc' in _CACHE:
        return _CACHE['nc']
    from contextlib import ExitStack
    import concourse.bacc as bacc
    import concourse.mybir as mybir
    import concourse.tile as tile
    from concourse.alu_op_type import AluOpType

    f32 = mybir.dt.float32
    bf16 = mybir.dt.bfloat16
    EXP = mybir.ActivationFunctionType.Exp

    nc = bacc.Bacc("TRN2", debug=False, enable_asserts=False)
    xT_d = nc.dram_tensor("xT", [BPC, C, N], bf16, kind="ExternalInput").ap()
    wqkv_d = nc.dram_tensor("wqkv", [C, 3 * C], bf16, kind="ExternalInput").ap()
    wout_d = nc.dram_tensor("wout", [C, C], bf16, kind="ExternalInput").ap()
    bias_d = nc.dram_tensor("biasT", [HEADS, TP, NT, N], bf16,
                            kind="ExternalInput").ap()
    out_d = nc.dram_tensor("out", [BPC, N, C], f32, kind="ExternalOutput").ap()

    with tile.TileContext(nc) as tc, ExitStack() as ctx:
        persist = ctx.enter_context(tc.tile_pool(name="persist", bufs=1))
        xT_pool = ctx.enter_context(tc.tile_pool(name="xTp", bufs=8))
        bias_pool = ctx.enter_context(tc.tile_pool(name="biasp", bufs=6))
        eraw_pool = ctx.enter_context(tc.tile_pool(name="erp", bufs=6))
        e_pool = ctx.enter_context(tc.tile_pool(name="ep", bufs=6))
        o_pool = ctx.enter_context(tc.tile_pool(name="op", bufs=3))
        r_pool = ctx.enter_context(tc.tile_pool(name="rp", bufs=3))
        osb_pool = ctx.enter_context(tc.tile_pool(name="osbp", bufs=3))
        big_psum = ctx.enter_context(tc.tile_pool(name="bigp", bufs=3,
                                                  space="PSUM"))
        ot_psum = ctx.enter_context(tc.tile_pool(name="otp", bufs=1,
                                                 space="PSUM"))

        qkT, vsb, attnT, bias_sb = {}, {}, {}, {}
        wqkv_sb, wout_sb = [None] * CT, [None] * CT
        for b in range(BPC):
            for fi in range(CT):
                attnT[b, fi] = persist.tile(
                    [128, N], bf16, tag=f"attnT{b}_{fi}", name=f"attnT{b}_{fi}")

        xts = {b: [None] * CT for b in range(BPC)}

        # DMA order: wqkv + xT(b0) first (first qk matmul unblocks after
        # ~3MB), then bias h0/h1, then xT(b1) — the prefix is DMA-bound,
        # so only unit-1's inputs go up front.
        for ci in range(CT):
            w = persist.tile([128, 3 * C], bf16, tag=f"wqkv{ci}",
                             name=f"wqkv{ci}")
            nc.sync.dma_start(w, wqkv_d[ci * 128:(ci + 1) * 128, :])
            wqkv_sb[ci] = w
            xt = xT_pool.tile([128, N], bf16, tag="xT", name=f"xT0_{ci}")
            nc.sync.dma_start(xt, xT_d[0, ci * 128:(ci + 1) * 128, :])
            xts[0][ci] = xt

        def load_xt1():
            for ci in range(CT):
                xt = xT_pool.tile([128, N], bf16, tag="xT", name=f"xT1_{ci}")
                nc.sync.dma_start(xt, xT_d[1, ci * 128:(ci + 1) * 128, :])
                xts[1][ci] = xt

        def load_wout():
            for ci in range(CT):
                w = persist.tile([128, C], bf16, tag=f"wout{ci}",
                                 name=f"wout{ci}")
                nc.sync.dma_start(w, wout_d[ci * 128:(ci + 1) * 128, :])
                wout_sb[ci] = w

        def load_bias(h):
            bt = bias_pool.tile([TP, NT, N], bf16, tag="bias", name=f"bias{h}")
            nc.sync.dma_start(bt, bias_d[h])
            bias_sb[h] = bt

        def emit_qk_tile(b, ft, on_act):
            dst = persist.tile([128, N], bf16, tag=f"qkT{b}_{ft}",
                               name=f"qkT{b}_{ft}")
            qkT[b, ft] = dst
            ps = big_psum.tile([128, N], f32, tag="big", name=f"pj{b}_{ft}")
            for (c0w, cw) in CHUNKS:
                for ci in range(CT):
                    nc.tensor.matmul(
                        ps[:, c0w:c0w + cw],
                        wqkv_sb[ci][:, ft * 128:(ft + 1) * 128],
                        xts[b][ci][:, c0w:c0w + cw],
                        start=(ci == 0), stop=(ci == CT - 1))
            if on_act:
                nc.scalar.copy(dst, ps)     # prefix: ACT idle there
            else:
                nc.vector.tensor_copy(dst, ps)

        def emit_v_tile(b, t):
            vt = persist.tile([TP, HEADS, KD + 1], bf16, tag=f"v{b}_{t}",
                              name=f"v{b}_{t}")
            vsb[b, t] = vt
            ps = big_psum.tile([TP, C], f32, tag="big", name=f"pv{b}_{t}")
            for ci in range(CT):
                nc.tensor.matmul(
                    ps, xts[b][ci][:, t * TP:(t + 1) * TP],
                    wqkv_sb[ci][:, 2 * C:3 * C],
                    start=(ci == 0), stop=(ci == CT - 1))
            nc.vector.tensor_copy(
                vt[:, :, 0:KD], ps.rearrange("p (h d) -> p h d", h=HEADS))
            nc.vector.memset(vt[:, :, KD:KD + 1], 1.0)

        # deferred reciprocal-normalize of the previous unit (emitted where
        # the next unit's DVE stream has slack)
        pending_norm = [None]

        def flush_norm():
            if pending_norm[0] is not None:
                fn, pending_norm[0] = pending_norm[0], None
                fn()

        # ---- attention for one (head, batch) unit ----
        def attention_unit(h, b):
            hp, r0 = h // 2, (h % 2) * 64
            with nc.named_scope(f"attn{h}_{b}"):
                ot = ot_psum.tile([KD + 1, N], f32, tag="ot",
                                  name=f"ot{h}_{b}")
                esbs = {}

                def emit_vmm(kt):
                    for (c0w, cw) in CHUNKS:
                        nc.tensor.matmul(
                            ot[:, c0w:c0w + cw],
                            vsb[b, kt][:, h:h + 1, :],
                            esbs[kt][:, c0w:c0w + cw],
                            start=(kt == 0), stop=(kt == NT - 1))

                for kt in range(NT):
                    scp = big_psum.tile([TP, N], f32, tag="big",
                                        name=f"sc{h}_{b}_{kt}")
                    for (c0w, cw) in CHUNKS:
                        nc.tensor.matmul(
                            scp[:, c0w:c0w + cw],
                            qkT[b, 4 + hp][r0:r0 + 64, kt * TP:(kt + 1) * TP],
                            qkT[b, hp][r0:r0 + 64, c0w:c0w + cw],
                            start=True, stop=True)
                    # v-matmul for kt-1 AFTER scores(kt): the PE stream never
                    # parks behind ops whose ACT/DVE deps aren't done yet.
                    if kt >= 1:
                        emit_vmm(kt - 1)
                    if kt == 2:
                        flush_norm()
                    eraw = eraw_pool.tile([TP, N], bf16, tag="eraw",
                                          name=f"er{h}_{b}_{kt}")
                    nc.scalar.activation(eraw, scp, EXP)
                    esb = e_pool.tile([TP, N], bf16, tag="e",
                                      name=f"e{h}_{b}_{kt}")
                    nc.vector.tensor_tensor(
                        esb, eraw, bias_sb[h][:, kt, :], AluOpType.mult)
                    esbs[kt] = esb
                emit_vmm(NT - 1)
                # drain the accumulator to SBUF at once — frees the single
                # 'ot' PSUM slot after one DVE op instead of after the whole
                # reciprocal chain
                ofull = o_pool.tile([KD + 1, N], f32, tag="ofull",
                                    name=f"of{h}_{b}")
                nc.vector.tensor_copy(ofull, ot)

                def finish(ofull=ofull, b=b, hp=hp, r0=r0):
                    # custom-DVE recip must see base-partition 0 (its ucode
                    # does not honor a partition-offset src on HW), so hop
                    # the sum-exp row through a dedicated [1, N] tile first
                    srow = r_pool.tile([1, N], f32, tag="srow")
                    nc.vector.tensor_copy(srow, ofull[KD:KD + 1, :])
                    rrow = r_pool.tile([1, N], f32, tag="rrow")
                    nc.vector.reciprocal_approx_fast(rrow, srow)
                    rb = r_pool.tile([64, N], f32, tag="rb")
                    nc.gpsimd.partition_broadcast(rb, rrow)
                    nc.vector.tensor_tensor(
                        attnT[b, hp][r0:r0 + 64, :], ofull[0:KD, :], rb,
                        AluOpType.mult)
                pending_norm[0] = finish

        def emit_proj_tile(b, t, copy_on_act):
            ps = big_psum.tile([TP, C], f32, tag="big", name=f"pr{b}_{t}")
            for fi in range(CT):
                nc.tensor.matmul(
                    ps, attnT[b, fi][:, t * TP:(t + 1) * TP], wout_sb[fi],
                    start=(fi == 0), stop=(fi == CT - 1))
            osb = osb_pool.tile([TP, C], f32, tag="osb")
            if copy_on_act:
                nc.scalar.copy(osb, ps)
            else:
                nc.vector.tensor_copy(osb, ps)
            nc.sync.dma_start(out_d[b, t * TP:(t + 1) * TP, :], osb)

        # ---- master schedule ----
        load_bias(0)
        load_bias(1)
        load_xt1()
        with nc.named_scope("qkv_prefix"):
            emit_qk_tile(0, 0, on_act=True)
            emit_qk_tile(0, 4, on_act=True)
            for t in range(NT):
                emit_v_tile(0, t)
        load_wout()

        # units: heads 0-3 on b0, then b1; heads 4-7 on b0, then b1.
        units = ([(h, 0) for h in range(4)] + [(h, 1) for h in range(4)]
                 + [(h, 0) for h in range(4, 8)] + [(h, 1) for h in range(4, 8)])
        # PE filler + bias prefetch at each unit boundary (index = after
        # unit i+1), placed just-in-time for their consumer units.
        boundary = {
            0: [('qk', 0, 1), ('qk', 0, 5), ('bias', 2)],
            1: [('qk', 1, 0), ('v', 1, 0), ('v', 1, 1), ('bias', 3)],
            2: [('qk', 1, 4), ('v', 1, 2), ('v', 1, 3), ('bias', 4)],
            3: [('v', 1, 4), ('v', 1, 5), ('v', 1, 6), ('bias', 5)],
            4: [('qk', 1, 1), ('qk', 1, 5), ('bias', 6)],
            5: [('qk', 0, 2), ('qk', 0, 6), ('bias', 7)],
            6: [('qk', 1, 2)],
            7: [('qk', 0, 3), ('qk', 0, 7)],
            8: [('qk', 1, 6)],
            9: [('qk', 1, 3)],
            10: [('qk', 1, 7)],
            12: [('proj', 0, 0), ('proj', 0, 1)],
            13: [('proj', 0, 2), ('proj', 0, 3)],
            14: [('proj', 0, 4), ('proj', 0, 5), ('proj', 0, 6)],
        }
        for ui, (h, b) in enumerate(units):
            attention_unit(h, b)
            for f in boundary.get(ui, []):
                kind = f[0]
                if kind == 'qk':
                    with nc.named_scope(f"qk_fill_b{f[1]}_{f[2]}"):
                        emit_qk_tile(f[1], f[2], on_act=False)
                elif kind == 'v':
                    with nc.named_scope(f"v_fill_b{f[1]}_{f[2]}"):
                        emit_v_tile(f[1], f[2])
                elif kind == 'proj':
                    with nc.named_scope(f"proj_b{f[1]}_t{f[2]}"):
                        emit_proj_tile(f[1], f[2], copy_on_act=False)
                else:
                    load_bias(f[1])
        flush_norm()

        # ---- remaining output projection (tail) ----
        for t in range(NT):
            with nc.named_scope(f"proj_b1_t{t}"):
                emit_proj_tile(1, t, copy_on_act=(t % 2 == 0))

    nc.compile()
    _CACHE['nc'] = nc
    return nc


def host_prep(x, w_qkv, pos_table, w_out):
    x = np.asarray(x, np.float32).reshape(B, N, C)
    wq = np.array(np.asarray(w_qkv, np.float32), copy=True)
    wq[:, :C] *= np.float32(1.0 / np.sqrt(KD))
    wq_bf = wq.astype(ml_dtypes.bfloat16)
    idx = _rel_index()
    expb = np.exp(np.asarray(pos_table, np.float32)[:, idx].transpose(0, 2, 1))
    # [h, k, q] -> [h, p, kt, q] with k = kt*112 + p
    biasT = np.ascontiguousarray(
        expb.reshape(HEADS, NT, TP, N).transpose(0, 2, 1, 3)
    ).astype(ml_dtypes.bfloat16)
    wout = np.ascontiguousarray(np.asarray(w_out, np.float32)).astype(
        ml_dtypes.bfloat16)
    in_maps = []
    for c in range(NCORES):
        xT = np.ascontiguousarray(
            x[c * BPC:(c + 1) * BPC].transpose(0, 2, 1)).astype(
                ml_dtypes.bfloat16)  # [2, 512, 784]
        in_maps.append({"xT": xT, "wqkv": wq_bf, "wout": wout, "biasT": biasT})
    return in_maps


def run(in_maps, trace=False, trace_cores=None):
    import concourse.bass_utils as bass_utils
    nc = build_nc()
    return bass_utils.run_bass_kernel_spmd(
        nc, in_maps, core_ids=list(range(NCORES)),
        trace=trace, trace_cores=trace_cores)


def kernel(x, w_qkv, pos_table, w_out):
    in_maps = host_prep(x, w_qkv, pos_table, w_out)
    res = run(in_maps)
    out = np.stack([r["out"] for r in res.results])    # [8, 2, 784, 512]
    return np.ascontiguousarray(out.reshape(B, HH, WW, C)).astype(np.float32)
